# revision 40
# baseline (speedup 1.0000x reference)
"""FewShotSegmentation Trainium2 kernel (v3: fp16 datapath).

Math: for each batch b (one per NeuronCore):
  num[k, c]  = sum_{p: mask[p]==k+1} F[c, p]          (masked pooling, K=16)
  seg[p']    = argmax_k  (num[k,:] . q[:, p']) / ||num[k,:]||
The reference's den (pixel count) and query-norm cancel inside the argmax
(positive per-k / per-p' scales), and the eps clamp never binds, so neither
is computed.

v3 strategy (per core): the fp32 baseline was PE-bound (fp32 matmuls lower
to LOW_HIGH pairs at ~4 cyc/col, plus 256 PE transposes of F). fp16
quantization of F/q/num flips ~9/32768 argmax pixels (rel err ~1e-2, under
the 2e-2 gate even with truncation rounding), so:

  loads:    SWDGE cast-DMA fp32->fp16 (HBM traffic unchanged, SBUF halved;
            verified exact-RNE). Single SWDGE queue => sf chunks drain
            before qf groups at full HBM rate. All 16 emissions issued
            first at high priority; fstg bufs=5 so no emission ever waits
            (a stalled emission lets qf slip ahead in the SWDGE queue).
  S:        fp16 PE transposes (1 cyc/col vs 4 for fp32), 4 tiles batched
            per fp16 PSUM tile, alternating DVE/ACT copies into
            S[128p, 32j, c]. (xbar DMA-transpose was tried: right layout,
            ~430 GB/s port rate, but walrus serializes every DMA_TRANSPOSE
            against all other DMA traffic -- net loss.)
  pooling:  num_half (16k, 512c) += onehot_j.T @ S[:, j, half] over j
            (64 fp16 matmuls, N=512); halves gate on chunks 0-3 / 4-7.
  match:    dots (16k, 512p') += numT_i.T @ Qh_i (64 fp16 matmuls), ACT
            applies 1/||num|| scale, 4 small PE transposes per group ->
            (128p', 16k), DVE max/max_index argmax.

Walrus in this toolchain allows only ONE sync-wait per lowered instruction
for several instruction structs; _hoist_excess_matmul_waits post-processes
the scheduled module, moving excess waits onto wait-only event-semaphore
instructions.
"""

from contextlib import ExitStack

import numpy as np

import concourse.bass as bass
import concourse.mybir as mybir
import concourse.tile as tile
from concourse import masks
from concourse.bass_utils import run_bass_kernel_spmd

B, C, H, W = 8, 1024, 64, 64
P = H * W          # 4096 pixels
K = 16             # foreground classes
PART = 128
NCH = C // PART    # 8 channel chunks
NPJ = P // PART    # 32 pixel chunks
NG = 8             # query column groups
GW = P // NG       # 512 pixels per group
CHALF = C // 2     # pooling half width (one PSUM bank)

F32 = mybir.dt.float32
F16 = mybir.dt.float16
I32 = mybir.dt.int32
U32 = mybir.dt.uint32


def build_nc():
    nc = bass.Bass(target_bir_lowering=False)

    sf = nc.dram_tensor("sf", [C, P], F32, kind="ExternalInput")
    sm = nc.dram_tensor("sm", [P], I32, kind="ExternalInput")
    qf = nc.dram_tensor("qf", [C, P], F32, kind="ExternalInput")
    seg = nc.dram_tensor("seg", [P], I32, kind="ExternalOutput")

    with ExitStack() as ctx:
        tc = ctx.enter_context(tile.TileContext(nc))
        singles = ctx.enter_context(tc.tile_pool(name="singles", bufs=1))

        identity = singles.tile([PART, PART], F32)
        idh = singles.tile([PART, PART], F16)
        classvec_i = singles.tile([PART, K], I32)
        classvec = singles.tile([PART, K], F32)

        def build_consts():
            masks.make_identity(nc, idh[:])
            masks.make_identity(nc, identity[:])
            # classvec[p, k] = k+1 for every partition
            nc.gpsimd.iota(
                classvec_i[:], pattern=[[1, K]], base=1, channel_multiplier=0
            )
            nc.vector.tensor_copy(classvec[:], classvec_i[:])

        # one-hot masks: onehot[p, j, k] = (sm[j*128+p] == k+1), fp16 for
        # the pooling matmuls. Mask loaded contiguously (32, 128) and
        # transposed on PE.
        mask_nm_i = singles.tile([NPJ, PART], I32)
        mask_nmf = singles.tile([NPJ, PART], F32)
        mask_pm = singles.tile([PART, NPJ], F32)
        onehot = singles.tile([PART, NPJ, K], F32)
        onehot_h = singles.tile([PART, NPJ, K], F16)

        def build_onehot(misc_ps):
            nc.scalar.dma_start(
                out=mask_nm_i[:], in_=sm.rearrange("(n p) -> n p", p=PART)
            )
            nc.vector.tensor_copy(mask_nmf[:], mask_nm_i[:])
            mtr = misc_ps.tile([PART, NPJ], F32, tag="mtr")
            nc.tensor.transpose(mtr[:], mask_nmf[:], identity[:NPJ, :NPJ])
            nc.vector.tensor_copy(mask_pm[:], mtr[:])
            for j in range(NPJ):
                nc.vector.tensor_scalar(
                    onehot[:, j, :],
                    classvec[:],
                    mask_pm[:, j : j + 1],
                    None,
                    op0=mybir.AluOpType.is_equal,
                )
            nc.vector.tensor_copy(onehot_h[:], onehot[:])

        # pooled prototypes: k-major fp32 and c-major fp16 forms
        numK = singles.tile([K, C], F32)          # (16, 1024)
        numT = singles.tile([PART, NCH, K], F16)  # c-major (128,16) per chunk
        inv = singles.tile([K, 1], F32)
        nrm2 = singles.tile([K, NCH], F32)
        nrm = singles.tile([K, 1], F32)
        outt = singles.tile([PART, NPJ], F32)
        seg_sb = singles.tile([NPJ, PART], I32)

        # transposed support features, full resolution: S[p, j, c]
        S = singles.tile([PART, NPJ, C], F16)     # 64 KiB/partition

        def epi_copy(sel, out, in_):
            if sel % 2 == 0:
                nc.vector.tensor_copy(out, in_)
            else:
                nc.scalar.copy(out, in_)

        with (
            tc.tile_pool(name="fstg", bufs=NCH) as fstg,
            tc.tile_pool(name="qpool", bufs=NG) as qpool,
            tc.tile_pool(name="scp", bufs=2) as scpool,
            tc.tile_pool(name="sqp", bufs=1) as sqpool,
            tc.tile_pool(name="dtsb", bufs=4) as dtsbpool,
            tc.tile_pool(name="m8", bufs=4) as m8pool,
            tc.tile_pool(name="mi", bufs=4) as mipool,
            tc.tile_pool(name="dtr", bufs=2, space=bass.MemorySpace.PSUM) as dtrpool,
            tc.tile_pool(name="mps", bufs=1, space=bass.MemorySpace.PSUM) as misc_ps,
        ):
            # All SWDGE emissions first (high priority): the single SWDGE
            # queue drains them in emission order, so sf fully precedes qf
            # at full HBM rate.
            ftiles, qtiles = [], []
            with tc.high_priority():
                for i in range(NCH):
                    Fh = fstg.tile([PART, P], F16, name="Fh")
                    if i == 0:
                        # quarter-split: the first pixel-block transposes
                        # gate on a 512KB completion instead of 2MB
                        for q4 in range(4):
                            nc.gpsimd.dma_start(
                                out=Fh[:, 1024 * q4 : 1024 * (q4 + 1)],
                                in_=sf[:PART, 1024 * q4 : 1024 * (q4 + 1)],
                            )
                    else:
                        nc.gpsimd.dma_start(
                            out=Fh[:], in_=sf[PART * i : PART * (i + 1), :]
                        )
                    ftiles.append(Fh)
                # consts on the gpsimd queue between sf and qf emissions:
                # idh must be ready when chunk 0 lands (~14us), and the 8
                # qf emissions (~1.2us each) would push it past that.
                build_consts()
                gspecs = [(GW * g, GW) for g in range(NG)]
                for off, w in gspecs:
                    Q = qpool.tile([PART, NCH, GW], F16, name="Q")
                    nc.gpsimd.dma_start(
                        out=Q[:],
                        in_=qf.rearrange("(n p) q -> p n q", p=PART)[
                            :, :, off : off + w
                        ],
                    )
                    qtiles.append(Q)

            build_onehot(misc_ps)

            def numt_chain(ii):
                # c-major numT (fp16) + squared sums for one chunk of C
                dtr = dtrpool.tile([PART, K], F32)
                nc.tensor.transpose(
                    dtr[:],
                    numK[:, PART * ii : PART * (ii + 1)],
                    identity[:K, :K],
                )
                epi_copy(ii, numT[:, ii, :], dtr[:])
                sqs = sqpool.tile([K, PART], F32, tag="sq")
                nc.scalar.square(sqs[:], numK[:, PART * ii : PART * (ii + 1)])
                nc.vector.reduce_sum(
                    nrm2[:, ii : ii + 1], sqs[:], axis=mybir.AxisListType.X
                )

            # ---------------- transpose + pooling ----------------
            with (
                tc.tile_pool(name="pst", bufs=3, space=bass.MemorySpace.PSUM) as pspool,
                tc.tile_pool(name="pnum", bufs=1, space=bass.MemorySpace.PSUM) as pnpool,
            ):
                pn = [None, None]

                def pool_quad(h, jb):
                    # 4 pooling matmuls (N=512) for pixel blocks jb*4..jb*4+3
                    for t in range(4):
                        j = jb * 4 + t
                        nc.tensor.matmul(
                            pn[h][:],
                            lhsT=onehot_h[:, j, :],
                            rhs=S[:, j, CHALF * h : CHALF * (h + 1)],
                            start=(j == 0),
                            stop=(j == NPJ - 1),
                            skip_group_check=True,
                        )

                NB = NPJ // 4
                for i in range(NCH):
                    Fh = ftiles[i]
                    # half h's pooling interleaves (lag-1) into its LAST
                    # chunk's transpose stream: pool MM j needs S[:, j, half]
                    # complete, i.e. this chunk's j-block copied.
                    h = i // (NCH // 2)
                    last = i % (NCH // 2) == NCH // 2 - 1
                    if last:
                        pn[h] = pnpool.tile([K, CHALF], F32, name=f"pn{h}")
                    # fp16 PE transposes: S[p, j, 128i + c] = Fh[c, j*128+p]
                    for jb in range(NB):
                        pst = pspool.tile([PART, 4, PART], F16)
                        for t in range(4):
                            j = jb * 4 + t
                            nc.tensor.transpose(
                                pst[:, t, :],
                                Fh[:, PART * j : PART * (j + 1)],
                                idh[:],
                            )
                        epi_copy(
                            jb,
                            S[:, jb * 4 : jb * 4 + 4, PART * i : PART * (i + 1)],
                            pst[:],
                        )
                        if last and jb >= 1:
                            pool_quad(h, jb - 1)
                    if last:
                        pool_quad(h, NB - 1)
                        nc.scalar.copy(
                            numK[:, CHALF * h : CHALF * (h + 1)], pn[h][:]
                        )
                        for ii in range(NCH // 2 * h, NCH // 2 * (h + 1)):
                            numt_chain(ii)

            nc.vector.reduce_sum(nrm[:], nrm2[:], axis=mybir.AxisListType.X)
            nc.scalar.sqrt(nrm[:], nrm[:])
            nc.vector.reciprocal(inv[:], nrm[:])

            # ---------------- match phase ----------------
            with (
                tc.tile_pool(name="pdot", bufs=4, space=bass.MemorySpace.PSUM) as pdpool,
                tc.tile_pool(name="ops", bufs=1, space=bass.MemorySpace.PSUM) as otr_ps,
            ):
                def post_group(off, w, sck):
                    j0 = off // PART
                    nj = w // PART
                    for t in range(nj):
                        dtr = dtrpool.tile([PART, K], F32)
                        nc.tensor.transpose(
                            dtr[:],
                            sck[:, PART * t : PART * (t + 1)],
                            identity[:K, :K],
                        )
                        dt = dtsbpool.tile([PART, K], F32)
                        nc.vector.tensor_copy(dt[:], dtr[:])
                        m8 = m8pool.tile([PART, 8], F32)
                        nc.vector.max(m8[:], dt[:])
                        mi = mipool.tile([PART, 8], U32)
                        nc.vector.max_index(mi[:], m8[:], dt[:])
                        nc.vector.tensor_copy(outt[:, j0 + t : j0 + t + 1], mi[:, 0:1])

                pending = None
                for g, (off, w) in enumerate(gspecs):
                    Q = qtiles[g]
                    pd = pdpool.tile([K, GW], F32)
                    for i in range(NCH):
                        nc.tensor.matmul(
                            pd[:, :w],
                            lhsT=numT[:, i, :],
                            rhs=Q[:, i, :w],
                            start=(i == 0),
                            stop=(i == NCH - 1),
                        )
                    # scale by 1/||num|| (per-partition) while leaving PSUM
                    sck = scpool.tile([K, GW], F32, tag="sck")
                    nc.scalar.mul(sck[:, :w], pd[:, :w], inv[:])
                    if pending is not None:
                        post_group(*pending)
                    pending = (off, w, sck)
                post_group(*pending)
                # transpose the f32 index results and store seg contiguously
                otr = otr_ps.tile([NPJ, PART], F32, tag="otr")
                nc.tensor.transpose(otr[:], outt[:], identity[:])
                nc.vector.tensor_copy(seg_sb[:], otr[:])

            nc.scalar.dma_start(
                out=seg.rearrange("(n p) -> n p", p=PART), in_=seg_sb[:]
            )

    _hoist_excess_matmul_waits(nc)
    return nc


def _hoist_excess_matmul_waits(nc):
    """walrus allows only one sync-wait per lowered instruction for some
    instruction structs (fp32 matmul LW, pseudo-DMA, ...); hoist extras
    onto wait-only event-semaphore instructions inserted right before
    the instruction on the same queue."""
    n = 0
    for f in nc.m.functions:
        for bb in f.blocks:
            out, changed = [], False
            for ins in bb.instructions:
                w = list(ins.sync_info.on_wait) if ins.sync_info else []
                if len(w) >= 2:
                    for x in w[:-1]:
                        n += 1
                        out.append(
                            mybir.InstEventSemaphore(
                                name=f"I-wh-{n}",
                                engine=ins.engine,
                                ins=[],
                                outs=[],
                                sync_info=mybir.SyncInfo(on_wait=[x], on_update=[]),
                            )
                        )
                    ins.sync_info = mybir.SyncInfo(
                        on_wait=[w[-1]], on_update=list(ins.sync_info.on_update)
                    )
                    changed = True
                out.append(ins)
            if changed:
                bb.instructions = out


_NC_CACHE = None


def _get_nc():
    global _NC_CACHE
    if _NC_CACHE is None:
        _NC_CACHE = build_nc()
    return _NC_CACHE


def run(inputs: dict, trace: bool = False, **kw):
    """Shard over batch, run on 8 cores, gather. Returns (seg, BassKernelResults)."""
    sf = np.ascontiguousarray(inputs["support_features"], dtype=np.float32)
    sm = np.ascontiguousarray(inputs["support_masks"], dtype=np.int32)
    qf = np.ascontiguousarray(inputs["query_features"], dtype=np.float32)
    assert sf.shape == (B, C, H, W), sf.shape
    assert sm.shape == (B, 1, H, W), sm.shape
    assert qf.shape == (B, C, H, W), qf.shape

    in_maps = [
        {
            "sf": sf[b].reshape(C, P),
            "sm": sm[b].reshape(P),
            "qf": qf[b].reshape(C, P),
        }
        for b in range(B)
    ]
    res = run_bass_kernel_spmd(
        _get_nc(), in_maps, core_ids=list(range(B)), trace=trace, **kw
    )
    seg = np.stack([res.results[b]["seg"] for b in range(B)]).reshape(B, H, W)
    return seg.astype(np.int32), res


def kernel(**inputs) -> np.ndarray:
    seg, _ = run(inputs, trace=False)
    return seg


# revision 41
# speedup vs baseline: 1.1717x; 1.1717x over previous
"""FewShotSegmentation Trainium2 kernel (v3: fp16 datapath).

Math: for each batch b (one per NeuronCore):
  num[k, c]  = sum_{p: mask[p]==k+1} F[c, p]          (masked pooling, K=16)
  seg[p']    = argmax_k  (num[k,:] . q[:, p']) / ||num[k,:]||
The reference's den (pixel count) and query-norm cancel inside the argmax
(positive per-k / per-p' scales), and the eps clamp never binds, so neither
is computed.

v3 strategy (per core): the fp32 baseline was PE-bound (fp32 matmuls lower
to LOW_HIGH pairs at ~4 cyc/col, plus 256 PE transposes of F). fp16
quantization of F/q/num flips ~9/32768 argmax pixels (rel err ~1e-2, under
the 2e-2 gate even with truncation rounding), so:

  loads:    SWDGE cast-DMA fp32->fp16 (HBM traffic unchanged, SBUF halved;
            verified exact-RNE). Single SWDGE queue => sf chunks drain
            before qf groups at full HBM rate. All 16 emissions issued
            first at high priority; fstg bufs=5 so no emission ever waits
            (a stalled emission lets qf slip ahead in the SWDGE queue).
  S:        fp16 PE transposes (1 cyc/col vs 4 for fp32), 4 tiles batched
            per fp16 PSUM tile, alternating DVE/ACT copies into
            S[128p, 32j, c]. (xbar DMA-transpose was tried: right layout,
            ~430 GB/s port rate, but walrus serializes every DMA_TRANSPOSE
            against all other DMA traffic -- net loss.)
  pooling:  num_half (16k, 512c) += onehot_j.T @ S[:, j, half] over j
            (64 fp16 matmuls, N=512); halves gate on chunks 0-3 / 4-7.
  match:    dots (16k, 512p') += numT_i.T @ Qh_i (64 fp16 matmuls), ACT
            applies 1/||num|| scale, 4 small PE transposes per group ->
            (128p', 16k), DVE max/max_index argmax.

Walrus in this toolchain allows only ONE sync-wait per lowered instruction
for several instruction structs; _hoist_excess_matmul_waits post-processes
the scheduled module, moving excess waits onto wait-only event-semaphore
instructions.
"""

from contextlib import ExitStack

import numpy as np

import concourse.bass as bass
import concourse.mybir as mybir
import concourse.tile as tile
from concourse import masks
from concourse.bass_utils import run_bass_kernel_spmd

B, C, H, W = 8, 1024, 64, 64
P = H * W          # 4096 pixels
K = 16             # foreground classes
PART = 128
NCH = C // PART    # 8 channel chunks
NPJ = P // PART    # 32 pixel chunks
NG = 8             # query column groups
GW = P // NG       # 512 pixels per group
CHALF = C // 2     # pooling half width (one PSUM bank)

F32 = mybir.dt.float32
F16 = mybir.dt.float16
I32 = mybir.dt.int32
U32 = mybir.dt.uint32


def build_nc():
    nc = bass.Bass(target_bir_lowering=False)

    sf = nc.dram_tensor("sf", [C, P], F32, kind="ExternalInput")
    sm = nc.dram_tensor("sm", [P], I32, kind="ExternalInput")
    qf = nc.dram_tensor("qf", [C, P], F32, kind="ExternalInput")
    seg = nc.dram_tensor("seg", [P], I32, kind="ExternalOutput")

    with ExitStack() as ctx:
        tc = ctx.enter_context(tile.TileContext(nc))
        singles = ctx.enter_context(tc.tile_pool(name="singles", bufs=1))

        identity = singles.tile([PART, PART], F32)
        idh = singles.tile([PART, PART], F16)
        classvec_i = singles.tile([PART, K], I32)
        classvec = singles.tile([PART, K], F32)

        def build_consts():
            masks.make_identity(nc, idh[:])
            masks.make_identity(nc, identity[:])
            # classvec[p, k] = k+1 for every partition
            nc.gpsimd.iota(
                classvec_i[:], pattern=[[1, K]], base=1, channel_multiplier=0
            )
            nc.vector.tensor_copy(classvec[:], classvec_i[:])

        # one-hot masks: onehot[p, j, k] = (sm[j*128+p] == k+1), fp16 for
        # the pooling matmuls. Mask loaded contiguously (32, 128) and
        # transposed on PE.
        mask_nm_i = singles.tile([NPJ, PART], I32)
        mask_nmf = singles.tile([NPJ, PART], F32)
        mask_pm = singles.tile([PART, NPJ], F32)
        onehot = singles.tile([PART, NPJ, K], F32)
        onehot_h = singles.tile([PART, NPJ, K], F16)

        def build_onehot(misc_ps):
            nc.scalar.dma_start(
                out=mask_nm_i[:], in_=sm.rearrange("(n p) -> n p", p=PART)
            )
            nc.vector.tensor_copy(mask_nmf[:], mask_nm_i[:])
            mtr = misc_ps.tile([PART, NPJ], F32, tag="mtr")
            nc.tensor.transpose(mtr[:], mask_nmf[:], identity[:NPJ, :NPJ])
            nc.vector.tensor_copy(mask_pm[:], mtr[:])
            for j in range(NPJ):
                nc.vector.tensor_scalar(
                    onehot[:, j, :],
                    classvec[:],
                    mask_pm[:, j : j + 1],
                    None,
                    op0=mybir.AluOpType.is_equal,
                )
            nc.vector.tensor_copy(onehot_h[:], onehot[:])

        # pooled prototypes: k-major fp32 and c-major fp16 forms
        numK = singles.tile([K, C], F32)          # (16, 1024)
        numT = singles.tile([PART, NCH, K], F16)  # c-major (128,16) per chunk
        inv = singles.tile([K, 1], F32)
        nrm2 = singles.tile([K, NCH], F32)
        nrm = singles.tile([K, 1], F32)
        outt = singles.tile([PART, NPJ], F32)
        seg_sb = singles.tile([NPJ, PART], I32)

        # transposed support features, full resolution: S[p, j, c]
        S = singles.tile([PART, NPJ, C], F16)     # 64 KiB/partition

        def epi_copy(sel, out, in_):
            if sel % 2 == 0:
                nc.vector.tensor_copy(out, in_)
            else:
                nc.scalar.copy(out, in_)

        with (
            tc.tile_pool(name="fstg", bufs=NCH) as fstg,
            tc.tile_pool(name="qpool", bufs=NG - 1) as qpool,
            tc.tile_pool(name="qspool", bufs=4) as qspool,
            tc.tile_pool(name="scp", bufs=2) as scpool,
            tc.tile_pool(name="sqp", bufs=1) as sqpool,
            tc.tile_pool(name="dtsb", bufs=4) as dtsbpool,
            tc.tile_pool(name="m8", bufs=4) as m8pool,
            tc.tile_pool(name="mi", bufs=4) as mipool,
            tc.tile_pool(name="dtr", bufs=2, space=bass.MemorySpace.PSUM) as dtrpool,
            tc.tile_pool(name="mps", bufs=1, space=bass.MemorySpace.PSUM) as misc_ps,
        ):
            # All SWDGE emissions first (high priority): the single SWDGE
            # queue drains them in emission order, so sf fully precedes qf
            # at full HBM rate.
            ftiles, qtiles = [], []
            with tc.high_priority():
                for i in range(NCH):
                    Fh = fstg.tile([PART, P], F16, name="Fh")
                    if i == 0:
                        # quarter-split: the first pixel-block transposes
                        # gate on a 512KB completion instead of 2MB
                        for q4 in range(4):
                            nc.gpsimd.dma_start(
                                out=Fh[:, 1024 * q4 : 1024 * (q4 + 1)],
                                in_=sf[:PART, 1024 * q4 : 1024 * (q4 + 1)],
                            )
                    else:
                        nc.gpsimd.dma_start(
                            out=Fh[:], in_=sf[PART * i : PART * (i + 1), :]
                        )
                    ftiles.append(Fh)
                # consts on the gpsimd queue between sf and qf emissions:
                # idh must be ready when chunk 0 lands (~14us), and the 8
                # qf emissions (~1.2us each) would push it past that.
                build_consts()
                # last 512-group loaded as four 128-col sub-DMAs: the
                # match/argmax pipeline tracks their staggered arrival,
                # so only ONE slice's serial chain is exposed after the
                # final qf byte (instead of the whole group's)
                gspecs = [(GW * g, GW) for g in range(NG - 1)] + [
                    (GW * (NG - 1) + PART * s, PART) for s in range(4)
                ]
                for off, w in gspecs:
                    if w == GW:
                        Q = qpool.tile([PART, NCH, GW], F16, name="Q")
                    else:
                        Q = qspool.tile([PART, NCH, PART], F16, name="Qs")
                    nc.gpsimd.dma_start(
                        out=Q[:, :, :w],
                        in_=qf.rearrange("(n p) q -> p n q", p=PART)[
                            :, :, off : off + w
                        ],
                    )
                    qtiles.append(Q)

            build_onehot(misc_ps)

            def numt_chain(ii):
                # c-major numT (fp16) + squared sums for one chunk of C
                dtr = dtrpool.tile([PART, K], F32)
                nc.tensor.transpose(
                    dtr[:],
                    numK[:, PART * ii : PART * (ii + 1)],
                    identity[:K, :K],
                )
                epi_copy(ii, numT[:, ii, :], dtr[:])
                sqs = sqpool.tile([K, PART], F32, tag="sq")
                nc.scalar.square(sqs[:], numK[:, PART * ii : PART * (ii + 1)])
                nc.vector.reduce_sum(
                    nrm2[:, ii : ii + 1], sqs[:], axis=mybir.AxisListType.X
                )

            # ---------------- transpose + pooling ----------------
            with (
                tc.tile_pool(name="pst", bufs=3, space=bass.MemorySpace.PSUM) as pspool,
                tc.tile_pool(name="pnum", bufs=1, space=bass.MemorySpace.PSUM) as pnpool,
            ):
                pn = [None, None]

                def pool_quad(h, jb):
                    # 4 pooling matmuls (N=512) for pixel blocks jb*4..jb*4+3
                    for t in range(4):
                        j = jb * 4 + t
                        nc.tensor.matmul(
                            pn[h][:],
                            lhsT=onehot_h[:, j, :],
                            rhs=S[:, j, CHALF * h : CHALF * (h + 1)],
                            start=(j == 0),
                            stop=(j == NPJ - 1),
                            skip_group_check=True,
                        )

                NB = NPJ // 4
                for i in range(NCH):
                    Fh = ftiles[i]
                    # half h's pooling interleaves (lag-1) into its LAST
                    # chunk's transpose stream: pool MM j needs S[:, j, half]
                    # complete, i.e. this chunk's j-block copied.
                    h = i // (NCH // 2)
                    last = i % (NCH // 2) == NCH // 2 - 1
                    if last:
                        pn[h] = pnpool.tile([K, CHALF], F32, name=f"pn{h}")
                    # fp16 PE transposes: S[p, j, 128i + c] = Fh[c, j*128+p]
                    for jb in range(NB):
                        pst = pspool.tile([PART, 4, PART], F16)
                        for t in range(4):
                            j = jb * 4 + t
                            nc.tensor.transpose(
                                pst[:, t, :],
                                Fh[:, PART * j : PART * (j + 1)],
                                idh[:],
                            )
                        epi_copy(
                            jb,
                            S[:, jb * 4 : jb * 4 + 4, PART * i : PART * (i + 1)],
                            pst[:],
                        )
                        if last and jb >= 1:
                            pool_quad(h, jb - 1)
                    if last:
                        pool_quad(h, NB - 1)
                        nc.scalar.copy(
                            numK[:, CHALF * h : CHALF * (h + 1)], pn[h][:]
                        )
                        for ii in range(NCH // 2 * h, NCH // 2 * (h + 1)):
                            numt_chain(ii)

            nc.vector.reduce_sum(nrm[:], nrm2[:], axis=mybir.AxisListType.X)
            nc.scalar.sqrt(nrm[:], nrm[:])
            nc.vector.reciprocal(inv[:], nrm[:])

            # ---------------- match phase ----------------
            with (
                tc.tile_pool(name="pdot", bufs=4, space=bass.MemorySpace.PSUM) as pdpool,
                tc.tile_pool(name="ops", bufs=1, space=bass.MemorySpace.PSUM) as otr_ps,
            ):
                def post_group(off, w, sck):
                    j0 = off // PART
                    nj = w // PART
                    for t in range(nj):
                        dtr = dtrpool.tile([PART, K], F32)
                        nc.tensor.transpose(
                            dtr[:],
                            sck[:, PART * t : PART * (t + 1)],
                            identity[:K, :K],
                        )
                        dt = dtsbpool.tile([PART, K], F32)
                        nc.vector.tensor_copy(dt[:], dtr[:])
                        m8 = m8pool.tile([PART, 8], F32)
                        nc.vector.max(m8[:], dt[:])
                        mi = mipool.tile([PART, 8], U32)
                        nc.vector.max_index(mi[:], m8[:], dt[:])
                        nc.vector.tensor_copy(outt[:, j0 + t : j0 + t + 1], mi[:, 0:1])

                pending = None
                for g, (off, w) in enumerate(gspecs):
                    Q = qtiles[g]
                    pd = pdpool.tile([K, GW], F32)
                    for i in range(NCH):
                        nc.tensor.matmul(
                            pd[:, :w],
                            lhsT=numT[:, i, :],
                            rhs=Q[:, i, :w],
                            start=(i == 0),
                            stop=(i == NCH - 1),
                        )
                    # scale by 1/||num|| (per-partition) while leaving PSUM
                    sck = scpool.tile([K, GW], F32, tag="sck")
                    nc.scalar.mul(sck[:, :w], pd[:, :w], inv[:])
                    if pending is not None:
                        post_group(*pending)
                    pending = (off, w, sck)
                post_group(*pending)
                # transpose the f32 index results and store seg contiguously
                otr = otr_ps.tile([NPJ, PART], F32, tag="otr")
                nc.tensor.transpose(otr[:], outt[:], identity[:])
                nc.vector.tensor_copy(seg_sb[:], otr[:])

            nc.scalar.dma_start(
                out=seg.rearrange("(n p) -> n p", p=PART), in_=seg_sb[:]
            )

    _hoist_excess_matmul_waits(nc)
    return nc


def _hoist_excess_matmul_waits(nc):
    """walrus allows only one sync-wait per lowered instruction for some
    instruction structs (fp32 matmul LW, pseudo-DMA, ...); hoist extras
    onto wait-only event-semaphore instructions inserted right before
    the instruction on the same queue."""
    n = 0
    for f in nc.m.functions:
        for bb in f.blocks:
            out, changed = [], False
            for ins in bb.instructions:
                w = list(ins.sync_info.on_wait) if ins.sync_info else []
                if len(w) >= 2:
                    for x in w[:-1]:
                        n += 1
                        out.append(
                            mybir.InstEventSemaphore(
                                name=f"I-wh-{n}",
                                engine=ins.engine,
                                ins=[],
                                outs=[],
                                sync_info=mybir.SyncInfo(on_wait=[x], on_update=[]),
                            )
                        )
                    ins.sync_info = mybir.SyncInfo(
                        on_wait=[w[-1]], on_update=list(ins.sync_info.on_update)
                    )
                    changed = True
                out.append(ins)
            if changed:
                bb.instructions = out


_NC_CACHE = None


def _get_nc():
    global _NC_CACHE
    if _NC_CACHE is None:
        _NC_CACHE = build_nc()
    return _NC_CACHE


def run(inputs: dict, trace: bool = False, **kw):
    """Shard over batch, run on 8 cores, gather. Returns (seg, BassKernelResults)."""
    sf = np.ascontiguousarray(inputs["support_features"], dtype=np.float32)
    sm = np.ascontiguousarray(inputs["support_masks"], dtype=np.int32)
    qf = np.ascontiguousarray(inputs["query_features"], dtype=np.float32)
    assert sf.shape == (B, C, H, W), sf.shape
    assert sm.shape == (B, 1, H, W), sm.shape
    assert qf.shape == (B, C, H, W), qf.shape

    in_maps = [
        {
            "sf": sf[b].reshape(C, P),
            "sm": sm[b].reshape(P),
            "qf": qf[b].reshape(C, P),
        }
        for b in range(B)
    ]
    res = run_bass_kernel_spmd(
        _get_nc(), in_maps, core_ids=list(range(B)), trace=trace, **kw
    )
    seg = np.stack([res.results[b]["seg"] for b in range(B)]).reshape(B, H, W)
    return seg.astype(np.int32), res


def kernel(**inputs) -> np.ndarray:
    seg, _ = run(inputs, trace=False)
    return seg
